# revision 1
# baseline (speedup 1.0000x reference)
"""Causal single-head attention block for Trainium2, SPMD across 8 NeuronCores.

Problem (hardcoded):
    x:     [4, 2048, 1024] f32
    w_qkv: [1024, 3072]    f32   (q | k | v column blocks)
    w_out: [1024, 1024]    f32
    b_out: [1024]          f32
    y = softmax(causal(q @ k.T / 32)) @ v @ w_out + b_out     -> [4, 2048, 1024]

Sharding: 2 cores per batch element. Within a batch, the 16 query subtiles of
128 rows are dealt round-robin to the core pair (core parity h gets subtiles
s = 2k + h, k = 0..7) so both cores see the identical causal work profile
(key-chunk counts [1,1,2,2,3,3,4,4]) and a single SPMD program serves all 8
cores; per-core behavior differs only through input data (xT / gathered xQ /
mask tables). Each core computes K^T (SBUF-resident) and V (DRAM round-trip)
for the full 2048 keys of its batch, Q^T for its own 1024 queries, the
causal-masked softmax, attention-weighted values, and the output projection.

All matmuls run in float32r (TF32-like PE mode, 4x the fp32 matmul rate).
"""

import numpy as np

import concourse.mybir as mybir
import concourse.tile as tile
from concourse import bacc
from concourse.bass_utils import run_bass_kernel_spmd

FP32 = mybir.dt.float32
FP32R = mybir.dt.float32r
BF16 = mybir.dt.bfloat16
AF = mybir.ActivationFunctionType
ALU = mybir.AluOpType

B, S, D, NI, NO = 4, 2048, 1024, 1024, 1024
NCORES = 8
P = 128
DC = D // P    # 8 contraction chunks for the projections
IC = NI // P   # 8 inner-dim chunks
RC = S // 512  # 4 key/row production chunks
NSUB = 8       # local 128-row query subtiles per core
CC = [k // 2 + 1 for k in range(NSUB)]  # 512-key chunks per local subtile
SCALE = float(NI) ** -0.5
NEG = -1.0e9

_CACHED = {}


def _build():
    nc = bacc.Bacc(None, target_bir_lowering=False, debug=False, num_devices=NCORES)

    xT = nc.dram_tensor("xT", [D, S], FP32R, kind="ExternalInput").ap()
    xQ = nc.dram_tensor("xQ", [D, NSUB * P], FP32R, kind="ExternalInput").ap()
    wk_d = nc.dram_tensor("wk", [D, NI], FP32R, kind="ExternalInput").ap()
    wv_d = nc.dram_tensor("wv", [D, NI], FP32R, kind="ExternalInput").ap()
    wq_d = nc.dram_tensor("wq", [D, NI], FP32R, kind="ExternalInput").ap()
    wo_d = nc.dram_tensor("wo", [NI, NO], FP32R, kind="ExternalInput").ap()
    masks = nc.dram_tensor("masks", [NSUB, P, 512], BF16, kind="ExternalInput").ap()
    bb = nc.dram_tensor("bb", [P, NO], FP32, kind="ExternalInput").ap()
    ident = nc.dram_tensor("ident", [P, P], FP32R, kind="ExternalInput").ap()
    y = nc.dram_tensor("y", [NSUB * P, NO], FP32, kind="ExternalOutput").ap()

    with tile.TileContext(nc) as tc:
        with (
            tc.tile_pool(name="const", bufs=1) as constp,
            tc.tile_pool(name="ktpool", bufs=IC) as ktp,
            tc.tile_pool(name="qtpool", bufs=IC) as qtp,
            tc.tile_pool(name="accp", bufs=2, space="PSUM") as accp,
            tc.tile_pool(name="tpp", bufs=2, space="PSUM") as tpp,
            tc.tile_pool(name="opp", bufs=4, space="PSUM") as opp,
            tc.tile_pool(name="dram", bufs=1, space="DRAM") as dramp,
        ):
            ident_sb = constp.tile([P, P], FP32R, name="ident_sb", tag="id")
            nc.sync.dma_start(out=ident_sb[:], in_=ident[:])
            b_sb = constp.tile([P, NO], FP32, name="b_sb", tag="b")
            nc.sync.dma_start(out=b_sb[:], in_=bb[:])
            mask_sb = constp.tile([P, NSUB, 512], BF16, name="mask_sb", tag="mask")
            for k in range(NSUB):
                nc.sync.dma_start(out=mask_sb[:, k, :], in_=masks[k])

            KT = [ktp.tile([P, S], FP32R, name=f"kt{i}", tag="kt") for i in range(IC)]
            QT = [
                qtp.tile([P, NSUB * P], FP32R, name=f"qt{i}", tag="qt")
                for i in range(IC)
            ]
            v_dram = dramp.tile([S, NI], FP32R, name="v_dram", tag="vd")

            with tc.tile_pool(name="wpool", bufs=2 * DC) as wp:
                def load_w(src, label, nsplit):
                    ts = []
                    for d in range(DC):
                        t = wp.tile([P, NI], FP32R, name=f"{label}{d}", tag="w")
                        w_ = NI // nsplit
                        for q in range(nsplit):
                            nc.sync.dma_start(
                                out=t[:, w_ * q:w_ * (q + 1)],
                                in_=src[P * d:P * (d + 1), w_ * q:w_ * (q + 1)],
                            )
                        ts.append(t)
                    return ts

                with tc.tile_pool(name="xtp", bufs=12) as xtp:
                    def load_xt(rc):
                        ts = []
                        for d in range(DC):
                            t = xtp.tile([P, 512], FP32R, name=f"x{rc}_{d}", tag="xt")
                            for q in range(2):
                                nc.sync.dma_start(
                                    out=t[:, 256 * q:256 * (q + 1)],
                                    in_=xT[P * d:P * (d + 1),
                                           512 * rc + 256 * q:512 * rc + 256 * (q + 1)],
                                )
                            ts.append(t)
                        return ts

                    # ---- Phase 0: Q^T for all 1024 local queries ----
                    # wq/xq DMAs interleaved per d so the first psum's inputs
                    # arrive in consumption order.
                    wq = []
                    for qh in range(2):
                        xqs = []
                        for d in range(DC):
                            if qh == 0:
                                wt = wp.tile([P, NI], FP32R, name=f"wq{d}", tag="w")
                                for q in range(4):
                                    nc.sync.dma_start(
                                        out=wt[:, 256 * q:256 * (q + 1)],
                                        in_=wq_d[P * d:P * (d + 1),
                                                 256 * q:256 * (q + 1)],
                                    )
                                wq.append(wt)
                            t = xtp.tile([P, 512], FP32R, name=f"xq{qh}_{d}", tag="xt")
                            for q in range(2):
                                nc.sync.dma_start(
                                    out=t[:, 256 * q:256 * (q + 1)],
                                    in_=xQ[P * d:P * (d + 1),
                                           512 * qh + 256 * q:
                                           512 * qh + 256 * (q + 1)],
                                )
                            xqs.append(t)
                        if qh == 0:
                            wk = load_w(wk_d, "wk", 2)
                        for i in range(IC):
                            ps = accp.tile([P, 512], FP32, name="ps_qt", tag="acc")
                            for d in range(DC):
                                nc.tensor.matmul(
                                    ps[:], wq[d][:, P * i:P * (i + 1)], xqs[d][:],
                                    start=(d == 0), stop=(d == DC - 1),
                                )
                            nc.vector.tensor_copy(
                                QT[i][:, 512 * qh:512 * (qh + 1)], ps[:]
                            )
                    xt0 = load_xt(0)      # prefetch K/V chunk 0

                    # ---- Phase 1: K^T (SBUF-resident) and V (DRAM) ----
                    with tc.tile_pool(name="vst", bufs=4) as vstp:
                        wv = load_w(wv_d, "wv", 2)  # reuses wq's slots after Q^T
                        for rc in range(RC):
                            xts = xt0 if rc == 0 else load_xt(rc)
                            for i in range(IC):
                                ps = accp.tile([P, 512], FP32, name="ps_kt", tag="acc")
                                for d in range(DC):
                                    nc.tensor.matmul(
                                        ps[:], wk[d][:, P * i:P * (i + 1)], xts[d][:],
                                        start=(d == 0), stop=(d == DC - 1),
                                    )
                                nc.vector.tensor_copy(
                                    KT[i][:, 512 * rc:512 * (rc + 1)], ps[:]
                                )
                            for vs in range(4):
                                row = 512 * rc + P * vs
                                for ih in range(2):
                                    ps = accp.tile([P, 512], FP32, name="ps_v",
                                                   tag="acc")
                                    for d in range(DC):
                                        nc.tensor.matmul(
                                            ps[:],
                                            xts[d][:, P * vs:P * (vs + 1)],
                                            wv[d][:, 512 * ih:512 * (ih + 1)],
                                            start=(d == 0), stop=(d == DC - 1),
                                        )
                                    vt = vstp.tile([P, 512], FP32R, name="vstage",
                                                   tag="vst")
                                    nc.vector.tensor_copy(vt[:], ps[:])
                                    nc.sync.dma_start(
                                        out=v_dram[row:row + P,
                                                   512 * ih:512 * (ih + 1)],
                                        in_=vt[:],
                                    )

            # ---- attention, 4 pair-groups of 2 subtiles ----
            with tc.tile_pool(name="wopool", bufs=DC) as wop, \
                 tc.tile_pool(name="vfixp", bufs=4) as vfixp:
                wo = []
                for d in range(DC):
                    t = wop.tile([P, NI], FP32R, name=f"wo{d}", tag="wo")
                    for q in range(2):
                        nc.sync.dma_start(
                            out=t[:, 512 * q:512 * (q + 1)],
                            in_=wo_d[P * d:P * (d + 1), 512 * q:512 * (q + 1)],
                        )
                    wo.append(t)
                # V rows [0:512) are read by every group: pin them in SBUF
                vfix = []
                for t in range(4):
                    vf = vfixp.tile([P, NI], FP32R, name=f"vfix{t}", tag="vfix")
                    for q in range(2):
                        nc.sync.dma_start(
                            out=vf[:, 512 * q:512 * (q + 1)],
                            in_=v_dram[P * t:P * (t + 1), 512 * q:512 * (q + 1)],
                        )
                    vfix.append(vf)
                with (
                    tc.tile_pool(name="ppool", bufs=2) as ppool,
                    tc.tile_pool(name="ptpool", bufs=3) as ptpool,
                    tc.tile_pool(name="otpool", bufs=8) as otpool,
                    tc.tile_pool(name="vrd", bufs=4) as vrdp,
                    tc.tile_pool(name="ypool", bufs=2) as ypool,
                    tc.tile_pool(name="stp", bufs=4) as stp,
                ):
                    for g in range(4):
                        L = g + 1
                        k0, k1 = 2 * g, 2 * g + 1
                        Ps = {}
                        for k in (k0, k1):
                            p_t = ppool.tile([P, 4 * 512], FP32R, name=f"p{k}", tag="p")
                            sums = stp.tile([P, 4], FP32, name=f"sums{k}", tag="sums")
                            # diagonal chunk first: its mask+exp chain overlaps
                            # the remaining chunks' matmuls
                            for kc in ([L - 1] + list(range(L - 1))):
                                ps = accp.tile([P, 512], FP32, name="ps_sim", tag="acc")
                                for i in range(IC):
                                    nc.tensor.matmul(
                                        ps[:],
                                        QT[i][:, P * k:P * (k + 1)],
                                        KT[i][:, 512 * kc:512 * (kc + 1)],
                                        start=(i == 0), stop=(i == IC - 1),
                                    )
                                if kc == L - 1:
                                    nc.vector.tensor_tensor(
                                        out=ps[:], in0=ps[:], in1=mask_sb[:, k, :],
                                        op=ALU.add,
                                    )
                                nc.scalar.activation(
                                    p_t[:, 512 * kc:512 * (kc + 1)], ps[:], AF.Exp,
                                    scale=SCALE, accum_out=sums[:, kc:kc + 1],
                                )
                            ssum = stp.tile([P, 1], FP32, name=f"ssum{k}", tag="ss")
                            nc.vector.tensor_reduce(
                                ssum[:], sums[:, :L], axis=mybir.AxisListType.X,
                                op=ALU.add,
                            )
                            rsum = stp.tile([P, 1], FP32, name=f"rsum{k}", tag="rs")
                            nc.vector.reciprocal(rsum[:], ssum[:])
                            nc.vector.tensor_scalar_mul(
                                p_t[:, :512 * L], p_t[:, :512 * L], rsum[:]
                            )
                            Ps[k] = p_t

                        ops = [
                            opp.tile([P, 512], FP32, name=f"op{g}_{j}", tag="op")
                            for j in range(4)
                        ]
                        nt = 4 * L
                        for t in range(nt):
                            tp_ps = tpp.tile([P, 256], FP32R, name="tp", tag="tp")
                            nc.tensor.transpose(
                                tp_ps[:, 0:P], Ps[k0][:, P * t:P * (t + 1)], ident_sb[:]
                            )
                            nc.tensor.transpose(
                                tp_ps[:, P:256], Ps[k1][:, P * t:P * (t + 1)],
                                ident_sb[:]
                            )
                            pt_t = ptpool.tile([P, 256], FP32R, name="pt", tag="pt")
                            nc.vector.tensor_copy(pt_t[:], tp_ps[:])
                            if t < 4:
                                v_t = vfix[t]
                            else:
                                v_t = vrdp.tile([P, NI], FP32R, name="v_t", tag="v")
                                for q in range(2):
                                    nc.sync.dma_start(
                                        out=v_t[:, 512 * q:512 * (q + 1)],
                                        in_=v_dram[P * t:P * (t + 1),
                                                   512 * q:512 * (q + 1)],
                                    )
                            for m in range(IC):
                                # one accumulation group per PSUM bank: start
                                # only on the bank's first matmul (whole-bank
                                # pending-zero makes the sibling column-half's
                                # first write an overwrite), stop on its last
                                nc.tensor.matmul(
                                    ops[m // 2][:, 256 * (m % 2):256 * (m % 2) + 256],
                                    v_t[:, P * m:P * (m + 1)],
                                    pt_t[:],
                                    start=(t == 0 and m % 2 == 0),
                                    stop=(t == nt - 1 and m % 2 == 1),
                                )

                        oT = []
                        for m in range(IC):
                            ot = otpool.tile([P, 256], FP32R, name=f"ot{g}_{m}",
                                             tag="ot")
                            nc.vector.tensor_copy(
                                ot[:], ops[m // 2][:, 256 * (m % 2):256 * (m % 2) + 256]
                            )
                            oT.append(ot)

                        # ---- output projection for this group's 2 subtiles ----
                        # y psums cycle through the opp pool so accp stays free
                        # for the next group's sim matmuls
                        for col, k in enumerate((k0, k1)):
                            for oh in range(2):
                                ps = opp.tile([P, 512], FP32, name="ps_y", tag="op")
                                for i in range(IC):
                                    nc.tensor.matmul(
                                        ps[:],
                                        oT[i][:, P * col:P * (col + 1)],
                                        wo[i][:, 512 * oh:512 * (oh + 1)],
                                        start=(i == 0), stop=(i == IC - 1),
                                    )
                                y_sb = ypool.tile([P, 512], FP32, name="y_sb", tag="y")
                                nc.vector.tensor_tensor(
                                    out=y_sb[:], in0=ps[:],
                                    in1=b_sb[:, 512 * oh:512 * (oh + 1)], op=ALU.add,
                                )
                                nc.sync.dma_start(
                                    out=y[P * k:P * (k + 1), 512 * oh:512 * (oh + 1)],
                                    in_=y_sb[:],
                                )

    nc.compile()
    return nc


def _prep_inputs(x, w_qkv, w_out, b_out):
    import ml_dtypes
    x = np.asarray(x, dtype=np.float32)
    w_qkv = np.asarray(w_qkv, dtype=np.float32)
    w_out = np.asarray(w_out, dtype=np.float32)
    b_out = np.asarray(b_out, dtype=np.float32)

    wq = np.ascontiguousarray(w_qkv[:, 0 * NI:1 * NI])
    wk = np.ascontiguousarray(w_qkv[:, 1 * NI:2 * NI])
    wv = np.ascontiguousarray(w_qkv[:, 2 * NI:3 * NI])
    b_bcast = np.ascontiguousarray(np.broadcast_to(b_out[None, :], (P, NO)))
    ident = np.eye(P, dtype=np.float32)

    xTs = [np.ascontiguousarray(x[b].T) for b in range(B)]

    in_maps = []
    for c in range(NCORES):
        b, h = c // 2, c % 2
        subs = [2 * k + h for k in range(NSUB)]
        xQ = np.concatenate(
            [xTs[b][:, P * s:P * (s + 1)] for s in subs], axis=1
        )
        m = np.empty((NSUB, P, 512), dtype=ml_dtypes.bfloat16)
        cpos = np.arange(512)[None, :]
        prow = np.arange(P)[:, None]
        for k in range(NSUB):
            off = P * subs[k] - 512 * (CC[k] - 1)
            m[k] = np.where(cpos <= off + prow, 0.0, NEG)
        in_maps.append({
            "xT": xTs[b], "xQ": np.ascontiguousarray(xQ),
            "wk": wk, "wv": wv, "wq": wq, "wo": w_out,
            "masks": m, "bb": b_bcast, "ident": ident,
        })
    return in_maps


def _run(x, w_qkv, w_out, b_out, trace=False, **kw):
    if "nc" not in _CACHED:
        _CACHED["nc"] = _build()
    nc = _CACHED["nc"]
    in_maps = _prep_inputs(x, w_qkv, w_out, b_out)
    res = run_bass_kernel_spmd(nc, in_maps, list(range(NCORES)), trace=trace, **kw)
    out = np.empty((B, S, NO), dtype=np.float32)
    for c in range(NCORES):
        b, h = c // 2, c % 2
        yc = res.results[c]["y"]
        for k in range(NSUB):
            s = 2 * k + h
            out[b, P * s:P * (s + 1), :] = yc[P * k:P * (k + 1), :]
    return out, res


def kernel(x, w_qkv, w_out, b_out):
    out, _ = _run(x, w_qkv, w_out, b_out, trace=False)
    return out



# revision 2
# speedup vs baseline: 1.8963x; 1.8963x over previous
"""Causal single-head attention block for Trainium2, SPMD across 8 NeuronCores.

Problem (hardcoded):
    x:     [4, 2048, 1024] f32
    w_qkv: [1024, 3072]    f32   (q | k | v column blocks)
    w_out: [1024, 1024]    f32
    b_out: [1024]          f32
    y = softmax(causal(q @ k.T / 32)) @ v @ w_out + b_out     -> [4, 2048, 1024]

Algebraic folding (host-side, fp32):
    sim  = (x wq)(x wk)^T = x (wq wk^T) x^T          -> Mq  = wq @ wk.T
    out  = attn (x wv) wo = attn x (wv wo)           -> Wvo = wv @ w_out
so the device kernel never materializes Q/K/V: it computes
    QM^T = Mq^T x_q^T   (local queries only)
    sim  = QM x^T       (x^T streamed from DRAM, SBUF-resident)
    attnX = softmax(causal(sim)) @ x                 (x rows streamed)
    y    = attnX @ Wvo + b
This halves the tensor-engine work vs. projecting Q/K/V explicitly and
removes the duplicated K/V computation across the core pair.

Sharding: 2 cores per batch element. Within a batch, the 16 query subtiles of
128 rows are dealt round-robin to the core pair (core parity h gets subtiles
s = 2k + h, k = 0..7) so both cores see the identical causal work profile
(512-key chunk counts [1,1,2,2,3,3,4,4]) and a single SPMD program serves all
8 cores; per-core behavior differs only through input data.

All matmuls run in float32r (TF32-like PE mode, 4x the fp32 matmul rate).
"""

import numpy as np

import concourse.mybir as mybir
import concourse.tile as tile
from concourse import bacc
from concourse.bass_utils import run_bass_kernel_spmd

FP32 = mybir.dt.float32
FP32R = mybir.dt.float32r
BF16 = mybir.dt.bfloat16
AF = mybir.ActivationFunctionType
ALU = mybir.AluOpType

B, S, D, NI, NO = 4, 2048, 1024, 1024, 1024
NCORES = 8
P = 128
DC = D // P    # 8 contraction chunks for the projections
IC = NI // P   # 8 inner-dim chunks
RC = S // 512  # 4 key chunks
NSUB = 8       # local 128-row query subtiles per core
CC = [k // 2 + 1 for k in range(NSUB)]  # 512-key chunks per local subtile
SCALE = float(NI) ** -0.5
NEG = -1.0e9

_CACHED = {}


def _build():
    nc = bacc.Bacc(None, target_bir_lowering=False, debug=False, num_devices=NCORES)

    xT = nc.dram_tensor("xT", [D, S], FP32R, kind="ExternalInput").ap()
    xQ = nc.dram_tensor("xQ", [D, NSUB * P], FP32R, kind="ExternalInput").ap()
    xR = nc.dram_tensor("xR", [S, D], FP32R, kind="ExternalInput").ap()
    mq_d = nc.dram_tensor("mq", [D, D], FP32R, kind="ExternalInput").ap()
    wvo_d = nc.dram_tensor("wvo", [NI, NO], FP32R, kind="ExternalInput").ap()
    masks = nc.dram_tensor("masks", [NSUB, P, 512], BF16, kind="ExternalInput").ap()
    bb = nc.dram_tensor("bb", [P, NO], FP32, kind="ExternalInput").ap()
    ident = nc.dram_tensor("ident", [P, P], FP32R, kind="ExternalInput").ap()
    y = nc.dram_tensor("y", [NSUB * P, NO], FP32, kind="ExternalOutput").ap()

    with tile.TileContext(nc) as tc:
        with (
            tc.tile_pool(name="const", bufs=1) as constp,
            tc.tile_pool(name="xtpool", bufs=IC) as xtp,
            tc.tile_pool(name="qtpool", bufs=IC) as qtp,
            tc.tile_pool(name="accp", bufs=2, space="PSUM") as accp,
            tc.tile_pool(name="tpp", bufs=2, space="PSUM") as tpp,
            tc.tile_pool(name="opp", bufs=4, space="PSUM") as opp,
        ):
            ident_sb = constp.tile([P, P], FP32R, name="ident_sb", tag="id")
            nc.sync.dma_start(out=ident_sb[:], in_=ident[:])
            b_sb = constp.tile([P, NO], FP32, name="b_sb", tag="b")
            nc.sync.dma_start(out=b_sb[:], in_=bb[:])
            mask_sb = constp.tile([P, NSUB, 512], BF16, name="mask_sb", tag="mask")
            for k in range(NSUB):
                nc.sync.dma_start(out=mask_sb[:, k, :], in_=masks[k])

            # x^T for this batch, SBUF-resident: 8 tiles of [128 dims, 2048 keys]
            XT = [xtp.tile([P, S], FP32R, name=f"xt{i}", tag="xt") for i in range(IC)]
            QMT = [
                qtp.tile([P, NSUB * P], FP32R, name=f"qt{i}", tag="qt")
                for i in range(IC)
            ]

            # ---- Phase 0: QM^T = Mq^T @ xQ for all 1024 local queries ----
            # mq/xq DMAs interleaved per d so the first psum's inputs arrive in
            # consumption order.
            with tc.tile_pool(name="wpool", bufs=DC) as wp, \
                 tc.tile_pool(name="xqp", bufs=8) as xqp:
                mq = []
                for qh in range(2):
                    xqs = []
                    for d in range(DC):
                        if qh == 0:
                            wt = wp.tile([P, D], FP32R, name=f"mq{d}", tag="w")
                            for q in range(4):
                                nc.sync.dma_start(
                                    out=wt[:, 256 * q:256 * (q + 1)],
                                    in_=mq_d[P * d:P * (d + 1),
                                             256 * q:256 * (q + 1)],
                                )
                            mq.append(wt)
                        t = xqp.tile([P, 512], FP32R, name=f"xq{qh}_{d}", tag="xq")
                        for q in range(2):
                            nc.sync.dma_start(
                                out=t[:, 256 * q:256 * (q + 1)],
                                in_=xQ[P * d:P * (d + 1),
                                       512 * qh + 256 * q:
                                       512 * qh + 256 * (q + 1)],
                            )
                        xqs.append(t)
                    if qh == 0:
                        # x^T chunk 0 (first 512 keys) right behind the
                        # phase-0 operands so group 0's sim can start early
                        for i in range(IC):
                            nc.sync.dma_start(
                                out=XT[i][:, 0:512],
                                in_=xT[P * i:P * (i + 1), 0:512],
                            )
                    for i in range(IC):
                        ps = accp.tile([P, 512], FP32, name="ps_qt", tag="acc")
                        for d in range(DC):
                            nc.tensor.matmul(
                                ps[:], mq[d][:, P * i:P * (i + 1)], xqs[d][:],
                                start=(d == 0), stop=(d == DC - 1),
                            )
                        nc.vector.tensor_copy(
                            QMT[i][:, 512 * qh:512 * (qh + 1)], ps[:]
                        )

            # ---- attention, 4 pair-groups of 2 subtiles ----
            with tc.tile_pool(name="wopool", bufs=DC) as wop, \
                 tc.tile_pool(name="vfixp", bufs=4) as vfixp:
                # x rows [0:512) are read by every group: pin them in SBUF
                vfix = []
                for t in range(4):
                    vf = vfixp.tile([P, NI], FP32R, name=f"vfix{t}", tag="vfix")
                    for q in range(2):
                        nc.sync.dma_start(
                            out=vf[:, 512 * q:512 * (q + 1)],
                            in_=xR[P * t:P * (t + 1), 512 * q:512 * (q + 1)],
                        )
                    vfix.append(vf)
                wo = []
                for d in range(DC):
                    t = wop.tile([P, NO], FP32R, name=f"wo{d}", tag="wo")
                    for q in range(2):
                        nc.sync.dma_start(
                            out=t[:, 512 * q:512 * (q + 1)],
                            in_=wvo_d[P * d:P * (d + 1), 512 * q:512 * (q + 1)],
                        )
                    wo.append(t)
                # remaining x^T chunks (keys 512:2048) for groups 1-3
                for rc in range(1, RC):
                    for i in range(IC):
                        nc.sync.dma_start(
                            out=XT[i][:, 512 * rc:512 * (rc + 1)],
                            in_=xT[P * i:P * (i + 1), 512 * rc:512 * (rc + 1)],
                        )
                with (
                    tc.tile_pool(name="ppool", bufs=2) as ppool,
                    tc.tile_pool(name="ptpool", bufs=3) as ptpool,
                    tc.tile_pool(name="otpool", bufs=8) as otpool,
                    tc.tile_pool(name="vrd", bufs=4) as vrdp,
                    tc.tile_pool(name="ypool", bufs=2) as ypool,
                    tc.tile_pool(name="stp", bufs=4) as stp,
                ):
                    for g in range(4):
                        L = g + 1
                        k0, k1 = 2 * g, 2 * g + 1
                        Ps = {}
                        for k in (k0, k1):
                            p_t = ppool.tile([P, 4 * 512], FP32R, name=f"p{k}", tag="p")
                            sums = stp.tile([P, 4], FP32, name=f"sums{k}", tag="sums")
                            # diagonal chunk first: its mask+exp chain overlaps
                            # the remaining chunks' matmuls
                            for kc in ([L - 1] + list(range(L - 1))):
                                ps = accp.tile([P, 512], FP32, name="ps_sim", tag="acc")
                                for i in range(IC):
                                    nc.tensor.matmul(
                                        ps[:],
                                        QMT[i][:, P * k:P * (k + 1)],
                                        XT[i][:, 512 * kc:512 * (kc + 1)],
                                        start=(i == 0), stop=(i == IC - 1),
                                    )
                                if kc == L - 1:
                                    nc.vector.tensor_tensor(
                                        out=ps[:], in0=ps[:], in1=mask_sb[:, k, :],
                                        op=ALU.add,
                                    )
                                nc.scalar.activation(
                                    p_t[:, 512 * kc:512 * (kc + 1)], ps[:], AF.Exp,
                                    scale=SCALE, accum_out=sums[:, kc:kc + 1],
                                )
                            ssum = stp.tile([P, 1], FP32, name=f"ssum{k}", tag="ss")
                            nc.vector.tensor_reduce(
                                ssum[:], sums[:, :L], axis=mybir.AxisListType.X,
                                op=ALU.add,
                            )
                            rsum = stp.tile([P, 1], FP32, name=f"rsum{k}", tag="rs")
                            nc.vector.reciprocal(rsum[:], ssum[:])
                            nc.vector.tensor_scalar_mul(
                                p_t[:, :512 * L], p_t[:, :512 * L], rsum[:]
                            )
                            Ps[k] = p_t

                        ops = [
                            opp.tile([P, 512], FP32, name=f"op{g}_{j}", tag="op")
                            for j in range(4)
                        ]
                        nt = 4 * L
                        for t in range(nt):
                            tp_ps = tpp.tile([P, 256], FP32R, name="tp", tag="tp")
                            nc.tensor.transpose(
                                tp_ps[:, 0:P], Ps[k0][:, P * t:P * (t + 1)], ident_sb[:]
                            )
                            nc.tensor.transpose(
                                tp_ps[:, P:256], Ps[k1][:, P * t:P * (t + 1)],
                                ident_sb[:]
                            )
                            pt_t = ptpool.tile([P, 256], FP32R, name="pt", tag="pt")
                            nc.vector.tensor_copy(pt_t[:], tp_ps[:])
                            if t < 4:
                                v_t = vfix[t]
                            else:
                                v_t = vrdp.tile([P, NI], FP32R, name="v_t", tag="v")
                                for q in range(2):
                                    nc.sync.dma_start(
                                        out=v_t[:, 512 * q:512 * (q + 1)],
                                        in_=xR[P * t:P * (t + 1),
                                               512 * q:512 * (q + 1)],
                                    )
                            for m in range(IC):
                                # one accumulation group per PSUM bank: start
                                # only on the bank's first matmul (whole-bank
                                # pending-zero makes the sibling column-half's
                                # first write an overwrite), stop on its last
                                nc.tensor.matmul(
                                    ops[m // 2][:, 256 * (m % 2):256 * (m % 2) + 256],
                                    v_t[:, P * m:P * (m + 1)],
                                    pt_t[:],
                                    start=(t == 0 and m % 2 == 0),
                                    stop=(t == nt - 1 and m % 2 == 1),
                                )

                        oT = []
                        for m in range(IC):
                            ot = otpool.tile([P, 256], FP32R, name=f"ot{g}_{m}",
                                             tag="ot")
                            nc.vector.tensor_copy(
                                ot[:], ops[m // 2][:, 256 * (m % 2):256 * (m % 2) + 256]
                            )
                            oT.append(ot)

                        # ---- output projection for this group's 2 subtiles ----
                        # y psums cycle through the opp pool so accp stays free
                        # for the next group's sim matmuls
                        for col, k in enumerate((k0, k1)):
                            for oh in range(2):
                                ps = opp.tile([P, 512], FP32, name="ps_y", tag="op")
                                for i in range(IC):
                                    nc.tensor.matmul(
                                        ps[:],
                                        oT[i][:, P * col:P * (col + 1)],
                                        wo[i][:, 512 * oh:512 * (oh + 1)],
                                        start=(i == 0), stop=(i == IC - 1),
                                    )
                                y_sb = ypool.tile([P, 512], FP32, name="y_sb", tag="y")
                                nc.vector.tensor_tensor(
                                    out=y_sb[:], in0=ps[:],
                                    in1=b_sb[:, 512 * oh:512 * (oh + 1)], op=ALU.add,
                                )
                                nc.sync.dma_start(
                                    out=y[P * k:P * (k + 1), 512 * oh:512 * (oh + 1)],
                                    in_=y_sb[:],
                                )

    nc.compile()
    return nc


def _prep_inputs(x, w_qkv, w_out, b_out):
    import ml_dtypes
    x = np.asarray(x, dtype=np.float32)
    w_qkv = np.asarray(w_qkv, dtype=np.float32)
    w_out = np.asarray(w_out, dtype=np.float32)
    b_out = np.asarray(b_out, dtype=np.float32)

    wq = w_qkv[:, 0 * NI:1 * NI]
    wk = w_qkv[:, 1 * NI:2 * NI]
    wv = w_qkv[:, 2 * NI:3 * NI]
    mq = np.ascontiguousarray(wq @ wk.T)
    wvo = np.ascontiguousarray(wv @ w_out)
    b_bcast = np.ascontiguousarray(np.broadcast_to(b_out[None, :], (P, NO)))
    ident = np.eye(P, dtype=np.float32)

    xTs = [np.ascontiguousarray(x[b].T) for b in range(B)]

    in_maps = []
    for c in range(NCORES):
        b, h = c // 2, c % 2
        subs = [2 * k + h for k in range(NSUB)]
        xQc = np.concatenate(
            [xTs[b][:, P * s:P * (s + 1)] for s in subs], axis=1
        )
        m = np.empty((NSUB, P, 512), dtype=ml_dtypes.bfloat16)
        cpos = np.arange(512)[None, :]
        prow = np.arange(P)[:, None]
        for k in range(NSUB):
            off = P * subs[k] - 512 * (CC[k] - 1)
            m[k] = np.where(cpos <= off + prow, 0.0, NEG)
        in_maps.append({
            "xT": xTs[b], "xQ": np.ascontiguousarray(xQc), "xR": x[b],
            "mq": mq, "wvo": wvo,
            "masks": m, "bb": b_bcast, "ident": ident,
        })
    return in_maps


def _run(x, w_qkv, w_out, b_out, trace=False, **kw):
    if "nc" not in _CACHED:
        _CACHED["nc"] = _build()
    nc = _CACHED["nc"]
    in_maps = _prep_inputs(x, w_qkv, w_out, b_out)
    res = run_bass_kernel_spmd(nc, in_maps, list(range(NCORES)), trace=trace, **kw)
    out = np.empty((B, S, NO), dtype=np.float32)
    for c in range(NCORES):
        b, h = c // 2, c % 2
        yc = res.results[c]["y"]
        for k in range(NSUB):
            s = 2 * k + h
            out[b, P * s:P * (s + 1), :] = yc[P * k:P * (k + 1), :]
    return out, res


def kernel(x, w_qkv, w_out, b_out):
    out, _ = _run(x, w_qkv, w_out, b_out, trace=False)
    return out


# revision 4
# speedup vs baseline: 2.5700x; 1.3553x over previous
"""Causal single-head attention block for Trainium2, SPMD across 8 NeuronCores.

Problem (hardcoded):
    x:     [4, 2048, 1024] f32
    w_qkv: [1024, 3072]    f32   (q | k | v column blocks)
    w_out: [1024, 1024]    f32
    b_out: [1024]          f32
    y = softmax(causal(q @ k.T / 32)) @ v @ w_out + b_out     -> [4, 2048, 1024]

Algebraic folding (host-side, fp32):
    sim  = (x wq)(x wk)^T = x (wq wk^T) x^T          -> Mq  = wq @ wk.T
    out  = attn (x wv) wo = attn x (wv wo)           -> Wvo = wv @ w_out
so the device kernel never materializes Q/K/V: it computes
    QM^T = Mq^T x_q^T   (local queries only)
    sim  = QM x^T       (x^T SBUF-resident)
    attnX = softmax(causal(sim)) @ x                 (x rows streamed)
    y    = attnX @ Wvo + b
This halves the tensor-engine work vs. projecting Q/K/V explicitly and
removes the duplicated K/V computation across the core pair.

Sharding: 2 cores per batch element. Within a batch, the 16 query subtiles of
128 rows are dealt round-robin to the core pair (core parity h gets subtiles
s = 2k + h, k = 0..7) so both cores see the identical causal work profile
(512-key chunk counts [1,1,2,2,3,3,4,4]) and a single SPMD program serves all
8 cores; per-core behavior differs only through input data.

All matmul operands are bf16 (PSUM accumulation in fp32; softmax statistics
and the output stay fp32): the elementwise rounding step is 4x fp32r's, far
inside the tolerance, and bf16 enables fast weight load + halves DMA/DVE.
"""

import numpy as np

import concourse.mybir as mybir
import concourse.tile as tile
from concourse import bacc
from concourse.bass_utils import run_bass_kernel_spmd

FP32 = mybir.dt.float32
BF16 = mybir.dt.bfloat16
AF = mybir.ActivationFunctionType
ALU = mybir.AluOpType

B, S, D, NI, NO = 4, 2048, 1024, 1024, 1024
NCORES = 8
P = 128
DC = D // P    # 8 contraction chunks for the projections
IC = NI // P   # 8 inner-dim chunks
RC = S // 512  # 4 key chunks
NSUB = 8       # local 128-row query subtiles per core
CC = [k // 2 + 1 for k in range(NSUB)]  # 512-key chunks per local subtile
SCALE = float(NI) ** -0.5
NEG = -1.0e9

_CACHED = {}


def _build():
    nc = bacc.Bacc(None, target_bir_lowering=False, debug=False, num_devices=NCORES)

    xT = nc.dram_tensor("xT", [D, S], BF16, kind="ExternalInput").ap()
    xQ = nc.dram_tensor("xQ", [D, NSUB * P], BF16, kind="ExternalInput").ap()
    xR = nc.dram_tensor("xR", [S, D], BF16, kind="ExternalInput").ap()
    mq_d = nc.dram_tensor("mq", [D, D], BF16, kind="ExternalInput").ap()
    wvo_d = nc.dram_tensor("wvo", [NI, NO], BF16, kind="ExternalInput").ap()
    masks = nc.dram_tensor("masks", [NSUB, P, 512], BF16, kind="ExternalInput").ap()
    bb = nc.dram_tensor("bb", [P, NO], FP32, kind="ExternalInput").ap()
    ident = nc.dram_tensor("ident", [P, P], BF16, kind="ExternalInput").ap()
    y = nc.dram_tensor("y", [NSUB * P, NO], FP32, kind="ExternalOutput").ap()

    with tile.TileContext(nc) as tc:
        with (
            tc.tile_pool(name="const", bufs=1) as constp,
            tc.tile_pool(name="xtpool", bufs=IC) as xtp,
            tc.tile_pool(name="qtpool", bufs=IC) as qtp,
        ):
            XT = [xtp.tile([P, S], BF16, name=f"xt{i}", tag="xt") for i in range(IC)]
            QMT = [
                qtp.tile([P, NSUB * P], BF16, name=f"qt{i}", tag="qt")
                for i in range(IC)
            ]

            # ---- Phase 0: QM^T = Mq^T @ xQ for all 1024 local queries ----
            # d-outer accumulation into 8 concurrently-open PSUM banks: the
            # first matmul only needs mq[0]+xq[0] on chip, so the PE starts
            # ~1us in instead of waiting for the full Mq/xQ transfer.
            with tc.tile_pool(name="qacc", bufs=IC, space="PSUM") as qacc, \
                 tc.tile_pool(name="wpool", bufs=DC) as wp, \
                 tc.tile_pool(name="xqp", bufs=2 * DC) as xqp:
                mq = []
                xqs = [[], []]
                for d in range(DC):
                    wt = wp.tile([P, D], BF16, name=f"mq{d}", tag="w")
                    nc.sync.dma_start(out=wt[:], in_=mq_d[P * d:P * (d + 1), :])
                    mq.append(wt)
                    t = xqp.tile([P, 512], BF16, name=f"xq0_{d}", tag="xq")
                    nc.sync.dma_start(out=t[:], in_=xQ[P * d:P * (d + 1), 0:512])
                    xqs[0].append(t)
                for d in range(DC):
                    t = xqp.tile([P, 512], BF16, name=f"xq1_{d}", tag="xq")
                    nc.sync.dma_start(out=t[:], in_=xQ[P * d:P * (d + 1), 512:1024])
                    xqs[1].append(t)
                # x^T tiles (whole rows, one descriptor each) land well before
                # group 0's sim consumes chunk 0
                for i in range(IC):
                    nc.sync.dma_start(out=XT[i][:], in_=xT[P * i:P * (i + 1), :])
                mask_sb = constp.tile([P, NSUB, 512], BF16, name="mask_sb", tag="mask")
                for k in range(NSUB):
                    nc.sync.dma_start(out=mask_sb[:, k, :], in_=masks[k])
                ident_sb = constp.tile([P, P], BF16, name="ident_sb", tag="id")
                nc.sync.dma_start(out=ident_sb[:], in_=ident[:])

                for qh in range(2):
                    pss = [
                        qacc.tile([P, 512], FP32, name=f"ps_qt{qh}_{i}", tag="qa")
                        for i in range(IC)
                    ]
                    for d in range(DC):
                        for i in range(IC):
                            nc.tensor.matmul(
                                pss[i][:], mq[d][:, P * i:P * (i + 1)],
                                xqs[qh][d][:],
                                start=(d == 0), stop=(d == DC - 1),
                            )
                    for i in range(IC):
                        nc.vector.tensor_copy(
                            QMT[i][:, 512 * qh:512 * (qh + 1)], pss[i][:]
                        )

            # ---- attention, 4 pair-groups of 2 subtiles ----
            with (
                tc.tile_pool(name="accp", bufs=2, space="PSUM") as accp,
                tc.tile_pool(name="tpp", bufs=2, space="PSUM") as tpp,
                tc.tile_pool(name="opp", bufs=4, space="PSUM") as opp,
                tc.tile_pool(name="wopool", bufs=DC) as wop,
                tc.tile_pool(name="vfixp", bufs=4) as vfixp,
            ):
                # x rows [0:512) are read by every group: pin them in SBUF
                vfix = []
                for t in range(4):
                    vf = vfixp.tile([P, NI], BF16, name=f"vfix{t}", tag="vfix")
                    nc.sync.dma_start(out=vf[:], in_=xR[P * t:P * (t + 1), :])
                    vfix.append(vf)
                wo = []
                for d in range(DC):
                    t = wop.tile([P, NO], BF16, name=f"wo{d}", tag="wo")
                    nc.sync.dma_start(out=t[:], in_=wvo_d[P * d:P * (d + 1), :])
                    wo.append(t)
                b_sb = constp.tile([P, NO], FP32, name="b_sb", tag="b")
                nc.sync.dma_start(out=b_sb[:], in_=bb[:])
                with (
                    tc.tile_pool(name="ppool", bufs=3) as ppool,
                    tc.tile_pool(name="ptpool", bufs=3) as ptpool,
                    tc.tile_pool(name="otpool", bufs=2 * IC) as otpool,
                    tc.tile_pool(name="vrd", bufs=4) as vrdp,
                    tc.tile_pool(name="ypool", bufs=4) as ypool,
                    tc.tile_pool(name="stp", bufs=4) as stp,
                ):
                    for g in range(4):
                        L = g + 1
                        k0, k1 = 2 * g, 2 * g + 1
                        Ps = {}
                        for k in (k0, k1):
                            p_t = ppool.tile([P, 4 * 512], BF16, name=f"p{k}", tag="p")
                            sums = stp.tile([P, 4], FP32, name=f"sums{k}", tag="sums")
                            # diagonal chunk first: its mask+exp chain overlaps
                            # the remaining chunks' matmuls
                            for kc in ([L - 1] + list(range(L - 1))):
                                ps = accp.tile([P, 512], FP32, name="ps_sim", tag="acc")
                                for i in range(IC):
                                    nc.tensor.matmul(
                                        ps[:],
                                        QMT[i][:, P * k:P * (k + 1)],
                                        XT[i][:, 512 * kc:512 * (kc + 1)],
                                        start=(i == 0), stop=(i == IC - 1),
                                    )
                                if kc == L - 1:
                                    nc.vector.tensor_tensor(
                                        out=ps[:], in0=ps[:], in1=mask_sb[:, k, :],
                                        op=ALU.add,
                                    )
                                nc.scalar.activation(
                                    p_t[:, 512 * kc:512 * (kc + 1)], ps[:], AF.Exp,
                                    scale=SCALE, accum_out=sums[:, kc:kc + 1],
                                )
                            ssum = stp.tile([P, 1], FP32, name=f"ssum{k}", tag="ss")
                            nc.vector.tensor_reduce(
                                ssum[:], sums[:, :L], axis=mybir.AxisListType.X,
                                op=ALU.add,
                            )
                            rsum = stp.tile([P, 1], FP32, name=f"rsum{k}", tag="rs")
                            nc.vector.reciprocal(rsum[:], ssum[:])
                            nc.vector.tensor_scalar_mul(
                                p_t[:, :512 * L], p_t[:, :512 * L], rsum[:]
                            )
                            Ps[k] = p_t

                        ops = [
                            opp.tile([P, 512], FP32, name=f"op{g}_{j}", tag="op")
                            for j in range(4)
                        ]
                        nt = 4 * L
                        for t in range(nt):
                            tp_ps = tpp.tile([P, 256], BF16, name="tp", tag="tp")
                            nc.tensor.transpose(
                                tp_ps[:, 0:P], Ps[k0][:, P * t:P * (t + 1)], ident_sb[:]
                            )
                            nc.tensor.transpose(
                                tp_ps[:, P:256], Ps[k1][:, P * t:P * (t + 1)],
                                ident_sb[:]
                            )
                            pt_t = ptpool.tile([P, 256], BF16, name="pt", tag="pt")
                            nc.vector.tensor_copy(pt_t[:], tp_ps[:])
                            if t < 4:
                                v_t = vfix[t]
                            else:
                                v_t = vrdp.tile([P, NI], BF16, name="v_t", tag="v")
                                nc.sync.dma_start(
                                    out=v_t[:], in_=xR[P * t:P * (t + 1), :]
                                )
                            for m in range(IC):
                                # one accumulation group per PSUM bank: start
                                # only on the bank's first matmul (whole-bank
                                # pending-zero makes the sibling column-half's
                                # first write an overwrite), stop on its last
                                nc.tensor.matmul(
                                    ops[m // 2][:, 256 * (m % 2):256 * (m % 2) + 256],
                                    v_t[:, P * m:P * (m + 1)],
                                    pt_t[:],
                                    start=(t == 0 and m % 2 == 0),
                                    stop=(t == nt - 1 and m % 2 == 1),
                                )

                        oT = []
                        for m in range(IC):
                            ot = otpool.tile([P, 256], BF16, name=f"ot{g}_{m}",
                                             tag="ot")
                            nc.vector.tensor_copy(
                                ot[:], ops[m // 2][:, 256 * (m % 2):256 * (m % 2) + 256]
                            )
                            oT.append(ot)

                        # ---- output projection for this group's 2 subtiles ----
                        # y psums cycle through the opp pool so accp stays free
                        # for the next group's sim matmuls
                        for col, k in enumerate((k0, k1)):
                            for oh in range(2):
                                ps = opp.tile([P, 512], FP32, name="ps_y", tag="op")
                                for i in range(IC):
                                    nc.tensor.matmul(
                                        ps[:],
                                        oT[i][:, P * col:P * (col + 1)],
                                        wo[i][:, 512 * oh:512 * (oh + 1)],
                                        start=(i == 0), stop=(i == IC - 1),
                                    )
                                y_sb = ypool.tile([P, 512], FP32, name="y_sb", tag="y")
                                nc.vector.tensor_tensor(
                                    out=y_sb[:], in0=ps[:],
                                    in1=b_sb[:, 512 * oh:512 * (oh + 1)], op=ALU.add,
                                )
                                nc.sync.dma_start(
                                    out=y[P * k:P * (k + 1), 512 * oh:512 * (oh + 1)],
                                    in_=y_sb[:],
                                )

    nc.compile()
    return nc


def _prep_inputs(x, w_qkv, w_out, b_out):
    import ml_dtypes
    BF = ml_dtypes.bfloat16
    x = np.asarray(x, dtype=np.float32)
    w_qkv = np.asarray(w_qkv, dtype=np.float32)
    w_out = np.asarray(w_out, dtype=np.float32)
    b_out = np.asarray(b_out, dtype=np.float32)

    wq = w_qkv[:, 0 * NI:1 * NI]
    wk = w_qkv[:, 1 * NI:2 * NI]
    wv = w_qkv[:, 2 * NI:3 * NI]
    mq = np.ascontiguousarray((wq @ wk.T).astype(BF))
    wvo = np.ascontiguousarray((wv @ w_out).astype(BF))
    b_bcast = np.ascontiguousarray(np.broadcast_to(b_out[None, :], (P, NO)))
    ident = np.eye(P, dtype=BF)

    xbf = [x[b].astype(BF) for b in range(B)]
    xTs = [np.ascontiguousarray(xb.T) for xb in xbf]

    in_maps = []
    for c in range(NCORES):
        b, h = c // 2, c % 2
        subs = [2 * k + h for k in range(NSUB)]
        xQc = np.concatenate(
            [xTs[b][:, P * s:P * (s + 1)] for s in subs], axis=1
        )
        m = np.empty((NSUB, P, 512), dtype=BF)
        cpos = np.arange(512)[None, :]
        prow = np.arange(P)[:, None]
        for k in range(NSUB):
            off = P * subs[k] - 512 * (CC[k] - 1)
            m[k] = np.where(cpos <= off + prow, 0.0, NEG)
        in_maps.append({
            "xT": xTs[b], "xQ": np.ascontiguousarray(xQc), "xR": xbf[b],
            "mq": mq, "wvo": wvo,
            "masks": m, "bb": b_bcast, "ident": ident,
        })
    return in_maps


def _run(x, w_qkv, w_out, b_out, trace=False, **kw):
    if "nc" not in _CACHED:
        _CACHED["nc"] = _build()
    nc = _CACHED["nc"]
    in_maps = _prep_inputs(x, w_qkv, w_out, b_out)
    res = run_bass_kernel_spmd(nc, in_maps, list(range(NCORES)), trace=trace, **kw)
    out = np.empty((B, S, NO), dtype=np.float32)
    for c in range(NCORES):
        b, h = c // 2, c % 2
        yc = res.results[c]["y"]
        for k in range(NSUB):
            s = 2 * k + h
            out[b, P * s:P * (s + 1), :] = yc[P * k:P * (k + 1), :]
    return out, res


def kernel(x, w_qkv, w_out, b_out):
    out, _ = _run(x, w_qkv, w_out, b_out, trace=False)
    return out
